# revision 3
# baseline (speedup 1.0000x reference)
"""EventDrivenFusionFast fused kernel for 8 Trainium2 NeuronCores.

Strategy: pure batch-parallel SPMD (one batch element per core, B=8).
Per core, the whole network runs out of SBUF:
  trade/lob -> (cast bf16, xbar-transpose) -> event-impact matmul + LN + tanh
  -> has_trade mask -> DVE tensor_tensor_scan decay recurrence ->
  fusion MLPs (matmul + LN + gelu) -> q/k/v projections (transposed layout)
  -> attention with transposed scores (row-tiled QK^T over 4 heads,
     PSUM-wide exp on ACT, col-tiled attn@V + column-sum matmuls,
     reciprocal + K=1 broadcast matmuls for the softmax normalizer)
  -> out_proj + residual + LN -> output MLP (matmul + LN + gelu).

LN rstd is computed as exp(-0.5*ln(var+eps)) so every ACT op during the
attention stream lives in the natural_log_exp table set (no table thrash).
"""

import math

import numpy as np

import concourse.bass as bass
import concourse.mybir as mybir
import concourse.tile as tile
from concourse import bacc
from concourse.bass import ds, ts
from concourse.bass_utils import run_bass_kernel_spmd
from concourse.masks import make_identity
from concourse.tile_rust import add_dep_helper

B, T, D, NH = 8, 2048, 128, 4
HD = D // NH
C = T // 128          # 16 token chunks of 128
QB = 512              # q-block (PSUM bank width in fp32)
NQ = T // QB          # 4 q-blocks
KC = T // 128         # 16 k chunks of 128
EPS = 1e-5
SCALE = 1.0 / math.sqrt(HD)

F32 = mybir.dt.float32
BF16 = mybir.dt.bfloat16
Act = mybir.ActivationFunctionType
Op = mybir.AluOpType

_CACHE: dict[tuple, object] = {}


def _build(flags: frozenset):
    nontriv = lambda name: name in flags

    nc = bacc.Bacc(None, target_bir_lowering=False)

    # ------------------------------------------------------------- dram io
    lob = nc.dram_tensor("lob_feat", [T, D], F32, kind="ExternalInput")
    trade = nc.dram_tensor("trade_feat", [T, D], F32, kind="ExternalInput")
    ht = nc.dram_tensor("has_trade", [T, 1], F32, kind="ExternalInput")
    ei_w = nc.dram_tensor("ei_w", [D, D], F32, kind="ExternalInput")
    decay = nc.dram_tensor("decay", [D, 1], F32, kind="ExternalInput")
    in_proj_w = nc.dram_tensor("in_proj_w", [3 * D, D], F32, kind="ExternalInput")
    in_proj_b = nc.dram_tensor("in_proj_b", [3 * D, 1], F32, kind="ExternalInput")
    out_proj_w = nc.dram_tensor("out_proj_w", [D, D], F32, kind="ExternalInput")
    lob_w = nc.dram_tensor("lob_w", [D, 2 * D], F32, kind="ExternalInput")
    tr_w = nc.dram_tensor("tr_w", [D, 2 * D], F32, kind="ExternalInput")
    of_w = nc.dram_tensor("of_w", [D, 2 * D], F32, kind="ExternalInput")
    out_h = nc.dram_tensor("out", [T, D], F32, kind="ExternalOutput")

    vec_ins = {}
    for name in ("ei_b", "ei_g", "ei_beta", "out_proj_b", "cn_g", "cn_b",
                 "lob_b", "lob_g", "lob_beta", "tr_b", "tr_g", "tr_beta",
                 "of_b", "of_g", "of_beta", "v_b"):
        if nontriv(name):
            vec_ins[name] = nc.dram_tensor(name, [1, D], F32, kind="ExternalInput")

    with tile.TileContext(nc) as tc:
        import contextlib
        with contextlib.ExitStack() as ctx:
            cst = ctx.enter_context(tc.tile_pool(name="cst", bufs=1))
            wstage = ctx.enter_context(tc.tile_pool(name="wstage", bufs=2))
            big = ctx.enter_context(tc.tile_pool(name="big", bufs=1))
            stats = ctx.enter_context(tc.tile_pool(name="stats", bufs=1))
            scratch = ctx.enter_context(tc.tile_pool(name="scratch", bufs=3))
            expool = ctx.enter_context(tc.tile_pool(name="expool", bufs=3))
            rpool = ctx.enter_context(tc.tile_pool(name="rpool", bufs=2))
            ps_lin = ctx.enter_context(tc.tile_pool(name="ps_lin", bufs=2, space="PSUM"))
            ps_s = ctx.enter_context(tc.tile_pool(name="ps_s", bufs=1, space="PSUM"))
            ps_ctx = ctx.enter_context(tc.tile_pool(name="ps_ctx", bufs=1, space="PSUM"))
            ps_dr = ctx.enter_context(tc.tile_pool(name="ps_dr", bufs=1, space="PSUM"))

            # ------------------------------------------------ constants
            identity = cst.tile([128, 128], F32, tag="identity")
            make_identity(nc, identity)
            ones_col = cst.tile([128, 1], BF16, tag="ones_col")
            nc.vector.memset(ones_col, 1.0)
            ones_k1 = cst.tile([128, 32], BF16, tag="ones_k1")
            nc.vector.memset(ones_k1, 1.0)
            eps_t = cst.tile([128, 1], F32, tag="eps")
            nc.vector.memset(eps_t, EPS)

            bc_tiles = {}
            for name in vec_ins:
                t = cst.tile([128, D], F32, tag=f"bc_{name}")
                nc.gpsimd.dma_start(t, vec_ins[name].ap().to_broadcast([128, D]))
                bc_tiles[name] = t

            # ------------------------------------------------ weight prep (W^T as bf16)
            def prep_wT(src_ap, tag):
                st = wstage.tile([128, 128], F32, tag="wstage")
                nc.sync.dma_start(st, src_ap)
                p = ps_lin.tile([128, QB], F32, tag="lin")
                nc.tensor.transpose(p[:, :128], st, identity)
                wt = cst.tile([128, 128], BF16, tag=f"wT_{tag}")
                nc.vector.tensor_copy(wt, p[:, :128])
                return wt

            eiwT = prep_wT(ei_w[:, :], "eiw")
            wqT = prep_wT(in_proj_w[0:D, :], "wq")
            wkT = prep_wT(in_proj_w[D:2 * D, :], "wk")
            wvT = prep_wT(in_proj_w[2 * D:3 * D, :], "wv")
            opwT = prep_wT(out_proj_w[:, :], "opw")
            lobw1T = prep_wT(lob_w[:, 0:D], "lobw1")
            lobw2T = prep_wT(lob_w[:, D:2 * D], "lobw2")
            trw1T = prep_wT(tr_w[:, 0:D], "trw1")
            trw2T = prep_wT(tr_w[:, D:2 * D], "trw2")
            ofw1T = prep_wT(of_w[:, 0:D], "ofw1")
            ofw2T = prep_wT(of_w[:, D:2 * D], "ofw2")

            bq_t = cst.tile([128, 1], F32, tag="bq")
            nc.sync.dma_start(bq_t, in_proj_b[0:D, :])
            bk_t = cst.tile([128, 1], F32, tag="bk")
            nc.sync.dma_start(bk_t, in_proj_b[D:2 * D, :])

            # decay -> d = sigmoid(decay) = 0.5*tanh(0.5x) + 0.5 (avoids sigmoid table)
            d_raw = cst.tile([128, 1], F32, tag="d_raw")
            nc.sync.dma_start(d_raw, decay[:, :])
            d_tanh = cst.tile([128, 1], F32, tag="d_tanh")
            nc.scalar.activation(d_tanh, d_raw, Act.Tanh, scale=0.5)
            d_sig = cst.tile([128, 1], F32, tag="d_sig")
            nc.vector.tensor_scalar(d_sig, d_tanh, 0.5, 0.5, Op.mult, Op.add)
            d_full = cst.tile([128, T], F32, tag="d_full")
            nc.vector.memset(d_full, 1.0)
            nc.vector.tensor_scalar_mul(d_full, d_full, d_sig[:, :])

            ht_t = cst.tile([128, C], F32, tag="ht")
            nc.sync.dma_start(ht_t, ht.ap().rearrange("(c p) o -> p (c o)", p=128))

            # ------------------------------------------------ inputs: load, cast, transpose
            lob_n = big.tile([128, C, 128], F32, tag="lob_n")
            nc.sync.dma_start(lob_n, lob.ap().rearrange("(c p) d -> p c d", p=128))
            trade_n = big.tile([128, C, 128], F32, tag="trade_n")
            nc.sync.dma_start(trade_n, trade.ap().rearrange("(c p) d -> p c d", p=128))
            lob_nb = big.tile([128, C, 128], BF16, tag="lob_nb")
            nc.vector.tensor_copy(lob_nb, lob_n)
            trade_nb = big.tile([128, C, 128], BF16, tag="trade_nb")
            nc.vector.tensor_copy(trade_nb, trade_n)
            lobT = big.tile([128, T], BF16, tag="lobT")
            tradeT = big.tile([128, T], BF16, tag="tradeT")
            for c in range(C):
                nc.sync.dma_start_transpose(lobT[:, ds(c * 128, 128)], lob_nb[:, c, :])
                nc.sync.dma_start_transpose(tradeT[:, ds(c * 128, 128)], trade_nb[:, c, :])

            # ------------------------------------------------ LN helpers
            def stage_stats(x_f32, c, st_t, mv_t, pre_bias=None):
                """bn stats for chunk c of x_f32 [128, C, 128]."""
                if pre_bias is not None:
                    nc.vector.tensor_add(x_f32[:, c, :], x_f32[:, c, :], pre_bias)
                nc.vector.bn_stats(st_t[:, c, :], x_f32[:, c, :])
                nc.vector.bn_aggr(mv_t[:, c, :], st_t[:, c, :])

            def batched_rstd(mv_t, tag):
                """rstd [128, C] = exp(-0.5 * ln(var + eps)); negmr = -mean*rstd."""
                rstd = stats.tile([128, C], F32, tag=f"rstd_{tag}")
                nc.scalar.activation(rstd, mv_t[:, :, 1], Act.Ln, bias=eps_t[:, :])
                nc.scalar.activation(rstd, rstd, Act.Exp, scale=-0.5)
                negmr = stats.tile([128, C], F32, tag=f"negmr_{tag}")
                nc.vector.tensor_tensor(negmr, mv_t[:, :, 0], rstd, Op.mult)
                nc.vector.tensor_scalar_mul(negmr, negmr, -1.0)
                return rstd, negmr

            # ================================================ event impact
            x_ei = big.tile([128, C, 128], F32, tag="x_ei")
            st_ei = stats.tile([128, C, 6], F32, tag="st_ei")
            mv_ei = stats.tile([128, C, 2], F32, tag="mv_ei")
            for c in range(C):
                p = ps_lin.tile([128, QB], F32, tag="lin")
                nc.tensor.matmul(p[:, :128], tradeT[:, ds(c * 128, 128)], eiwT,
                                 start=True, stop=True)
                nc.scalar.copy(x_ei[:, c, :], p[:, :128])
                stage_stats(x_ei, c, st_ei, mv_ei,
                            pre_bias=bc_tiles.get("ei_b"))
            rstd_ei, negmr_ei = batched_rstd(mv_ei, "ei")
            impact_b = big.tile([128, C, 128], BF16, tag="impact_b")
            ei_applies = []
            for c in range(C):
                if nontriv("ei_g") or nontriv("ei_beta"):
                    tf = scratch.tile([128, 128], F32, tag="ei_tmp")
                    nc.vector.tensor_scalar(tf, x_ei[:, c, :],
                                            mv_ei[:, c, 0:1], rstd_ei[:, c:c + 1],
                                            Op.subtract, Op.mult)
                    if nontriv("ei_g"):
                        nc.vector.tensor_mul(tf, tf, bc_tiles["ei_g"])
                    if nontriv("ei_beta"):
                        nc.vector.tensor_add(tf, tf, bc_tiles["ei_beta"])
                    tb = scratch.tile([128, 128], BF16, tag="ei_tanh")
                    a = nc.scalar.activation(tb, tf, Act.Tanh)
                else:
                    tb = scratch.tile([128, 128], BF16, tag="ei_tanh")
                    a = nc.scalar.activation(tb, x_ei[:, c, :], Act.Tanh,
                                             bias=negmr_ei[:, c:c + 1],
                                             scale=rstd_ei[:, c:c + 1])
                ei_applies.append(a)
                nc.vector.tensor_scalar_mul(impact_b[:, c, :], tb, ht_t[:, c:c + 1])

            # transpose impact -> [D, T], then decay scan along T
            impT = big.tile([128, T], BF16, tag="impT")
            for c in range(C):
                nc.sync.dma_start_transpose(impT[:, ds(c * 128, 128)], impact_b[:, c, :])
            stateT = big.tile([128, T], BF16, tag="stateT")
            nc.vector.tensor_tensor_scan(stateT, d_full, impT, 0.0, Op.mult, Op.add)

            # ================================================ fusion MLPs
            def fusion_mlp(xT_first, w1T, w2T, pre_bias, g, beta, out_f32, out_b, tag):
                x_m = big.tile([128, C, 128], F32, tag=f"x_{tag}")
                st_m = stats.tile([128, C, 6], F32, tag=f"st_{tag}")
                mv_m = stats.tile([128, C, 2], F32, tag=f"mv_{tag}")
                for c in range(C):
                    p = ps_lin.tile([128, QB], F32, tag="lin")
                    nc.tensor.matmul(p[:, :128], xT_first[:, ds(c * 128, 128)], w1T,
                                     start=True, stop=False)
                    nc.tensor.matmul(p[:, :128], stateT[:, ds(c * 128, 128)], w2T,
                                     start=False, stop=True)
                    nc.scalar.copy(x_m[:, c, :], p[:, :128])
                    stage_stats(x_m, c, st_m, mv_m, pre_bias=pre_bias)
                rstd_m, negmr_m = batched_rstd(mv_m, tag)
                applies = []
                for c in range(C):
                    if g is not None or beta is not None:
                        tf = scratch.tile([128, 128], F32, tag=f"tmp_{tag}")
                        nc.vector.tensor_scalar(tf, x_m[:, c, :],
                                                mv_m[:, c, 0:1], rstd_m[:, c:c + 1],
                                                Op.subtract, Op.mult)
                        if g is not None:
                            nc.vector.tensor_mul(tf, tf, g)
                        if beta is not None:
                            nc.vector.tensor_add(tf, tf, beta)
                        dst = out_f32 if out_f32 is not None else out_b
                        a = nc.scalar.activation(
                            dst[:, c, :], tf, Act.Gelu)
                    else:
                        dst = out_f32 if out_f32 is not None else out_b
                        a = nc.scalar.activation(
                            dst[:, c, :], x_m[:, c, :], Act.Gelu,
                            bias=negmr_m[:, c:c + 1], scale=rstd_m[:, c:c + 1])
                    applies.append(a)
                return applies

            lobenh_n = big.tile([128, C, 128], F32, tag="lobenh_n")
            lob_applies = fusion_mlp(
                lobT, lobw1T, lobw2T, bc_tiles.get("lob_b"),
                bc_tiles.get("lob_g"), bc_tiles.get("lob_beta"),
                lobenh_n, None, "lob")
            trenh_b = big.tile([128, C, 128], BF16, tag="trenh_b")
            tr_applies = fusion_mlp(
                tradeT, trw1T, trw2T, bc_tiles.get("tr_b"),
                bc_tiles.get("tr_g"), bc_tiles.get("tr_beta"),
                None, trenh_b, "tr")

            lobenh_b = big.tile([128, C, 128], BF16, tag="lobenh_b")
            nc.vector.tensor_copy(lobenh_b, lobenh_n)
            lobenhT = big.tile([128, T], BF16, tag="lobenhT")
            trenhT = big.tile([128, T], BF16, tag="trenhT")
            for c in range(C):
                nc.sync.dma_start_transpose(lobenhT[:, ds(c * 128, 128)], lobenh_b[:, c, :])
                nc.sync.dma_start_transpose(trenhT[:, ds(c * 128, 128)], trenh_b[:, c, :])

            # ================================================ q/k/v projections
            QT = big.tile([128, T], BF16, tag="QT")
            KT = big.tile([128, T], BF16, tag="KT")
            for qc in range(NQ):
                p = ps_lin.tile([128, QB], F32, tag="lin")
                nc.tensor.matmul(p, wqT, lobenhT[:, ds(qc * QB, QB)], start=True, stop=True)
                nc.vector.tensor_scalar(QT[:, ds(qc * QB, QB)], p, bq_t[:, :], None, Op.add)
                p2 = ps_lin.tile([128, QB], F32, tag="lin")
                nc.tensor.matmul(p2, wkT, trenhT[:, ds(qc * QB, QB)], start=True, stop=True)
                nc.vector.tensor_scalar(KT[:, ds(qc * QB, QB)], p2, bk_t[:, :], None, Op.add)
            V_b = big.tile([128, C, 128], BF16, tag="V_b")
            for c in range(C):
                p = ps_lin.tile([128, QB], F32, tag="lin")
                nc.tensor.matmul(p[:, :128], trenhT[:, ds(c * 128, 128)], wvT,
                                 start=True, stop=True)
                if nontriv("v_b"):
                    nc.vector.tensor_add(p[:, :128], p[:, :128], bc_tiles["v_b"])
                nc.vector.tensor_copy(V_b[:, c, :], p[:, :128])

            # ================================================ attention + out_proj + of
            ctxT = big.tile([128, T], BF16, tag="ctxT")
            fused_b = big.tile([128, C, 128], BF16, tag="fused_b")
            fusedT = big.tile([128, T], BF16, tag="fusedT")
            x_of = big.tile([128, C, 128], F32, tag="x_of")
            st_cn = stats.tile([128, C, 6], F32, tag="st_cn")
            mv_cn = stats.tile([128, C, 2], F32, tag="mv_cn")
            st_of = stats.tile([128, C, 6], F32, tag="st_of")
            mv_of = stats.tile([128, C, 2], F32, tag="mv_of")
            rstd_cn = stats.tile([128, C], F32, tag="rstd_cn")
            rstd_of = stats.tile([128, C], F32, tag="rstd_of")
            negmr_of = stats.tile([128, C], F32, tag="negmr_of")

            exp_insts = []
            for qb in range(NQ):
                psum_ctx = ps_ctx.tile([128, QB], F32, tag="ctx")
                psum_den = ps_dr.tile([128, QB], F32, tag="dr")
                for kc in range(KC):
                    psum_sc = ps_s.tile([128, NH * QB], F32, tag="sc")
                    for h in range(NH):
                        nc.tensor.matmul(
                            psum_sc[:, ts(h, QB)],
                            KT[ds(HD * h, HD), ds(kc * 128, 128)],
                            QT[ds(HD * h, HD), ds(qb * QB, QB)],
                            start=True, stop=True,
                            tile_position=(HD * h, 0))
                    e_t = expool.tile([128, NH * QB], BF16, tag="exp")
                    ei_ = nc.scalar.activation(e_t, psum_sc, Act.Exp, scale=SCALE)
                    exp_insts.append(ei_)
                    for h in range(NH):
                        nc.tensor.matmul(
                            psum_ctx[ds(HD * h, HD), :],
                            V_b[:, kc, ds(HD * h, HD)],
                            e_t[:, ts(h, QB)],
                            start=(kc == 0), stop=(kc == KC - 1),
                            tile_position=(0, HD * h))
                    for h in range(NH):
                        nc.tensor.matmul(
                            psum_den[ds(HD * h, 1), :],
                            ones_col[:, :],
                            e_t[:, ts(h, QB)],
                            start=(kc == 0), stop=(kc == KC - 1),
                            tile_position=(0, HD * h))
                recip_b = rpool.tile([128, QB], BF16, tag="recip")
                with nc.allow_low_precision(reason="softmax denom recip in bf16"):
                    for h in range(NH):
                        nc.vector.reciprocal(recip_b[ds(HD * h, 1), :],
                                             psum_den[ds(HD * h, 1), :])
                psum_R = ps_dr.tile([128, QB], F32, tag="dr")
                for h in range(NH):
                    nc.tensor.matmul(
                        psum_R[ds(HD * h, HD), :],
                        ones_k1[ds(HD * h, 1), :],
                        recip_b[ds(HD * h, 1), :],
                        start=True, stop=True,
                        tile_position=(HD * h, HD * h))
                R_sb = rpool.tile([128, QB], F32, tag="R_sb")
                nc.vector.tensor_copy(R_sb, psum_R)
                nc.vector.tensor_tensor(ctxT[:, ds(qb * QB, QB)], psum_ctx, R_sb, Op.mult)

                # ---- out_proj + residual + cn-LN + of MMs for this q-block
                for j in range(NQ):
                    c = qb * NQ + j
                    p = ps_lin.tile([128, QB], F32, tag="lin")
                    nc.tensor.matmul(p[:, :128], ctxT[:, ds(c * 128, 128)], opwT,
                                     start=True, stop=True)
                    fp = scratch.tile([128, 128], F32, tag="fused_pre")
                    nc.vector.tensor_tensor(fp, p[:, :128], lobenh_n[:, c, :], Op.add)
                    if nontriv("out_proj_b"):
                        nc.vector.tensor_add(fp, fp, bc_tiles["out_proj_b"])
                    nc.vector.bn_stats(st_cn[:, c, :], fp)
                    nc.vector.bn_aggr(mv_cn[:, c, :], st_cn[:, c, :])
                    nc.scalar.activation(rstd_cn[:, c:c + 1], mv_cn[:, c, 1:2],
                                         Act.Ln, bias=eps_t[:, :])
                    nc.scalar.activation(rstd_cn[:, c:c + 1], rstd_cn[:, c:c + 1],
                                         Act.Exp, scale=-0.5)
                    if nontriv("cn_g") or nontriv("cn_b"):
                        tf = scratch.tile([128, 128], F32, tag="cn_tmp")
                        nc.vector.tensor_scalar(tf, fp, mv_cn[:, c, 0:1],
                                                rstd_cn[:, c:c + 1],
                                                Op.subtract, Op.mult)
                        if nontriv("cn_g"):
                            nc.vector.tensor_mul(tf, tf, bc_tiles["cn_g"])
                        if nontriv("cn_b"):
                            nc.vector.tensor_add(tf, tf, bc_tiles["cn_b"])
                        nc.vector.tensor_copy(fused_b[:, c, :], tf)
                    else:
                        nc.vector.tensor_scalar(fused_b[:, c, :], fp,
                                                mv_cn[:, c, 0:1], rstd_cn[:, c:c + 1],
                                                Op.subtract, Op.mult)
                    nc.sync.dma_start_transpose(fusedT[:, ds(c * 128, 128)],
                                                fused_b[:, c, :])
                    # of MMs + stats (gelu applied in the tail)
                    p2 = ps_lin.tile([128, QB], F32, tag="lin")
                    nc.tensor.matmul(p2[:, :128], fusedT[:, ds(c * 128, 128)], ofw1T,
                                     start=True, stop=False)
                    nc.tensor.matmul(p2[:, :128], trenhT[:, ds(c * 128, 128)], ofw2T,
                                     start=False, stop=True)
                    nc.vector.tensor_copy(x_of[:, c, :], p2[:, :128])
                    stage_stats(x_of, c, st_of, mv_of,
                                pre_bias=bc_tiles.get("of_b"))
                    nc.scalar.activation(rstd_of[:, c:c + 1], mv_of[:, c, 1:2],
                                         Act.Ln, bias=eps_t[:, :])
                    nc.scalar.activation(rstd_of[:, c:c + 1], rstd_of[:, c:c + 1],
                                         Act.Exp, scale=-0.5)
                    nc.vector.tensor_tensor(negmr_of[:, c:c + 1], mv_of[:, c, 0:1],
                                            rstd_of[:, c:c + 1], Op.mult)
                    nc.vector.tensor_scalar_mul(negmr_of[:, c:c + 1],
                                                negmr_of[:, c:c + 1], -1.0)

            # ---- tail: of gelu applies + output store
            out_f = big.tile([128, C, 128], F32, tag="out_f")
            last_exp = exp_insts[-1]
            for c in range(C):
                if nontriv("of_g") or nontriv("of_beta"):
                    tf = scratch.tile([128, 128], F32, tag="of_tmp")
                    nc.vector.tensor_scalar(tf, x_of[:, c, :], mv_of[:, c, 0:1],
                                            rstd_of[:, c:c + 1], Op.subtract, Op.mult)
                    if nontriv("of_g"):
                        nc.vector.tensor_mul(tf, tf, bc_tiles["of_g"])
                    if nontriv("of_beta"):
                        nc.vector.tensor_add(tf, tf, bc_tiles["of_beta"])
                    g = nc.scalar.activation(out_f[:, c, :], tf, Act.Gelu)
                else:
                    g = nc.scalar.activation(out_f[:, c, :], x_of[:, c, :], Act.Gelu,
                                             bias=negmr_of[:, c:c + 1],
                                             scale=rstd_of[:, c:c + 1])
                add_dep_helper(g.ins, last_exp.ins, sync=False,
                               reason="keep of-gelu out of the exp stream")
                nc.sync.dma_start(
                    out_h.ap().rearrange("(cc p) d -> p cc d", p=128)[:, c, :],
                    out_f[:, c, :])

    nc.compile()
    return nc


def _flags_from_inputs(inputs) -> frozenset:
    f = set()
    def nz(x):
        return not np.all(np.asarray(x) == 0.0)
    def nong1(x):
        return not np.all(np.asarray(x) == 1.0)
    if nz(inputs["ei_b"]): f.add("ei_b")
    if nong1(inputs["ei_g"]): f.add("ei_g")
    if nz(inputs["ei_beta"]): f.add("ei_beta")
    if nz(inputs["in_proj_b"][2 * D:3 * D]): f.add("v_b")
    if nz(inputs["out_proj_b"]): f.add("out_proj_b")
    if nong1(inputs["cn_g"]): f.add("cn_g")
    if nz(inputs["cn_b"]): f.add("cn_b")
    if nz(inputs["lob_b"]): f.add("lob_b")
    if nong1(inputs["lob_g"]): f.add("lob_g")
    if nz(inputs["lob_beta"]): f.add("lob_beta")
    if nz(inputs["tr_b"]): f.add("tr_b")
    if nong1(inputs["tr_g"]): f.add("tr_g")
    if nz(inputs["tr_beta"]): f.add("tr_beta")
    if nz(inputs["of_b"]): f.add("of_b")
    if nong1(inputs["of_g"]): f.add("of_g")
    if nz(inputs["of_beta"]): f.add("of_beta")
    return frozenset(f)


def kernel(**inputs) -> np.ndarray:
    inputs = {k: np.asarray(v) for k, v in inputs.items()}
    flags = _flags_from_inputs(inputs)
    if flags not in _CACHE:
        _CACHE[flags] = _build(flags)
    nc = _CACHE[flags]

    shared = {
        "ei_w": inputs["ei_w"].astype(np.float32),
        "decay": inputs["decay"].reshape(D, 1).astype(np.float32),
        "in_proj_w": inputs["in_proj_w"].astype(np.float32),
        "in_proj_b": inputs["in_proj_b"].reshape(3 * D, 1).astype(np.float32),
        "out_proj_w": inputs["out_proj_w"].astype(np.float32),
        "lob_w": inputs["lob_w"].astype(np.float32),
        "tr_w": inputs["tr_w"].astype(np.float32),
        "of_w": inputs["of_w"].astype(np.float32),
    }
    for name in ("ei_b", "ei_g", "ei_beta", "out_proj_b", "cn_g", "cn_b",
                 "lob_b", "lob_g", "lob_beta", "tr_b", "tr_g", "tr_beta",
                 "of_b", "of_g", "of_beta"):
        if name in flags:
            shared[name] = inputs[name].reshape(1, D).astype(np.float32)
    if "v_b" in flags:
        shared["v_b"] = inputs["in_proj_b"][2 * D:3 * D].reshape(1, D).astype(np.float32)

    in_maps = []
    for b in range(B):
        m = dict(shared)
        m["lob_feat"] = np.ascontiguousarray(inputs["lob_feat"][b], dtype=np.float32)
        m["trade_feat"] = np.ascontiguousarray(inputs["trade_feat"][b], dtype=np.float32)
        m["has_trade"] = np.ascontiguousarray(
            inputs["has_trade"][b].reshape(T, 1), dtype=np.float32)
        in_maps.append(m)

    res = run_bass_kernel_spmd(nc, in_maps, core_ids=list(range(B)))
    out = np.stack([res.results[b]["out"] for b in range(B)], axis=0)
    return out.astype(np.float32)


if __name__ == "__main__":
    rng = np.random.default_rng(0)
    ins = {
        "lob_feat": rng.standard_normal((B, T, D), dtype=np.float32),
        "trade_feat": rng.standard_normal((B, T, D), dtype=np.float32),
        "has_trade": (rng.random((B, T)) < 0.5).astype(np.float32),
        "ei_w": (rng.standard_normal((D, D), dtype=np.float32) / np.sqrt(D)),
        "ei_b": np.zeros(D, np.float32),
        "ei_g": np.ones(D, np.float32),
        "ei_beta": np.zeros(D, np.float32),
        "decay": np.full(D, 0.9, np.float32),
        "in_proj_w": (rng.standard_normal((3 * D, D), dtype=np.float32) / np.sqrt(D)),
        "in_proj_b": np.zeros(3 * D, np.float32),
        "out_proj_w": (rng.standard_normal((D, D), dtype=np.float32) / np.sqrt(D)),
        "out_proj_b": np.zeros(D, np.float32),
        "cn_g": np.ones(D, np.float32),
        "cn_b": np.zeros(D, np.float32),
        "lob_w": (rng.standard_normal((D, 2 * D), dtype=np.float32) / np.sqrt(2 * D)),
        "lob_b": np.zeros(D, np.float32),
        "lob_g": np.ones(D, np.float32),
        "lob_beta": np.zeros(D, np.float32),
        "tr_w": (rng.standard_normal((D, 2 * D), dtype=np.float32) / np.sqrt(2 * D)),
        "tr_b": np.zeros(D, np.float32),
        "tr_g": np.ones(D, np.float32),
        "tr_beta": np.zeros(D, np.float32),
        "of_w": (rng.standard_normal((D, 2 * D), dtype=np.float32) / np.sqrt(2 * D)),
        "of_b": np.zeros(D, np.float32),
        "of_g": np.ones(D, np.float32),
        "of_beta": np.zeros(D, np.float32),
    }
    out = kernel(**ins)
    print("kernel out", out.shape, out.dtype, np.abs(out).max())
